# revision 10
# baseline (speedup 1.0000x reference)
"""GemmaAttention (GQA, B=2 S=2048 HID=2048, 16 q-heads / 4 kv-heads, d=256)
on 8 Trainium2 NeuronCores.

Sharding: core = (batch b, head-group g) with b = core//4, g = core%4.
Each core computes q-heads [4g, 4g+4) and kv-head g (the reference's
repeat_kv quirk maps q-head h to kv-head h//4), producing a partial
o_proj output [S, HID] from its 1024 o_proj input features.  The host
sums the 4 partials per batch.  No collectives.

fp8 DoubleRow scheme: the Q/K/V projections, the score matmuls, and
o_proj run as fp8e4 matmuls in MatmulPerfMode.DoubleRow (2 k-tiles per
instruction at 0.5 cycles/row -> 4x bf16 per k-tile).  Full precision is
recovered by hi/lo splitting BOTH operands (x = fp8(x) + fp8(x - fp8(x)))
and accumulating the three cross terms hi*hi + lo*hi + hi*lo in PSUM
(the lo*lo term is ~0.06% and dropped) -> 0.75x bf16 cost at slightly
BETTER-than-bf16 accuracy.  hs and all weights are pre-split on the
host; q/k are split on-chip after rope (ACT writes hi, DVE writes the
lo residual); attention outputs are split after the rowsum normalize.
PV stays bf16 (probs quantization on-chip is not worth the DVE/ACT
traffic).  Weights are pre-scaled by WS=64 on the host so their fp8
encoding avoids denormals; cos/sin tables carry 1/WS to descale q/k,
v keeps the x64 (cancels in softmax normalization: oT comes out x64,
folded into the o_proj output descale 1/WS^2).

On-chip layout is "transposed" throughout: hsT [HID, S], qT/kT [d, S],
v natural [S, d], scores computed transposed [ks, qs].  Softmax skips
max-subtraction (score*scale is O(5), exp cannot overflow); 1/sqrt(d)
is folded into exp's scale immediate; the additive mask is accumulated
into the scores PSUM via an identity matmul.  In the causal variant,
diagonal ks-tiles are column-sliced to their live range [128m, 512) --
the triangle-mask matmul (N=128) opens the PSUM accumulation group.
Softmax denominators: GPSIMD accumulates probs tiles (acc +=) and does
the partition all-reduce, keeping both PE and DVE off the rowsum path.
"""

import sys

sys.path.insert(0, "/opt/trn_rl_repo")

import math

import numpy as np
import ml_dtypes

import concourse.bacc as bacc
import concourse.bass as bass
import concourse.bass_isa as bass_isa
import concourse.tile as tile
from concourse import mybir
from concourse.bass_utils import run_bass_kernel_spmd

B, S, HID = 2, 2048, 2048
N_HEADS, N_KV, HEAD_DIM = 16, 4, 256
HD2 = HEAD_DIM // 2  # 128
ROPE_BASE = 10000.0
P = 128
QB = 512  # qs block width (moving free dim)
NSB = S // QB  # 4 s-blocks
NHT = HID // P  # 16 hidden chunks
NKS = S // P  # 16 key tiles
HPC = N_HEADS // 4  # 4 q heads per core
FQ = HPC * HEAD_DIM  # 1024 q features per core
NFQ = FQ // P  # 8 oT partition tiles
SCALE = 1.0 / math.sqrt(HEAD_DIM)
WS = 64.0  # host-side weight pre-scale (fp8 denormal avoidance)
WSV = 32.0  # wv pre-scale: row-0 attention output is exactly v_0, and
# |v|*64 would graze fp8e4's 240 max; x32 keeps ~2x headroom

F32 = mybir.dt.float32
BF16 = mybir.dt.bfloat16
FP16 = mybir.dt.float16
F8 = mybir.dt.float8e4
NP_BF16 = ml_dtypes.bfloat16
NP_F8 = ml_dtypes.float8_e4m3
DR = mybir.MatmulPerfMode.DoubleRow
ACT_EXP = mybir.ActivationFunctionType.Exp
ACT_COPY = mybir.ActivationFunctionType.Copy


def _build(mask_mode: str):
    """mask_mode: 'causal' | 'none' | 'full'. Returns compiled Bacc."""
    nc = bacc.Bacc("TRN2", target_bir_lowering=False, debug=False, num_devices=8)

    hsT_hi = nc.dram_tensor("hsT_hi", [HID, S], F8, kind="ExternalInput").ap()
    hsT_lo = nc.dram_tensor("hsT_lo", [HID, S], F8, kind="ExternalInput").ap()
    wq_hi = nc.dram_tensor("wq_hi", [HID, FQ], F8, kind="ExternalInput").ap()
    wq_lo = nc.dram_tensor("wq_lo", [HID, FQ], F8, kind="ExternalInput").ap()
    wk_hi = nc.dram_tensor("wk_hi", [HID, HEAD_DIM], F8, kind="ExternalInput").ap()
    wk_lo = nc.dram_tensor("wk_lo", [HID, HEAD_DIM], F8, kind="ExternalInput").ap()
    wv_hi = nc.dram_tensor("wv_hi", [HID, HEAD_DIM], F8, kind="ExternalInput").ap()
    wv_lo = nc.dram_tensor("wv_lo", [HID, HEAD_DIM], F8, kind="ExternalInput").ap()
    wo_hi = nc.dram_tensor("wo_hi", [FQ, HID], F8, kind="ExternalInput").ap()
    wo_lo = nc.dram_tensor("wo_lo", [FQ, HID], F8, kind="ExternalInput").ap()
    cosT = nc.dram_tensor("cosT", [HD2, S], F32, kind="ExternalInput").ap()
    sinT = nc.dram_tensor("sinT", [HD2, S], F32, kind="ExternalInput").ap()
    if mask_mode == "causal":
        ident = nc.dram_tensor("ident", [P, P], BF16, kind="ExternalInput").ap()
        mtri = nc.dram_tensor("mtri", [P, P], BF16, kind="ExternalInput").ap()
    elif mask_mode == "full":
        ident = nc.dram_tensor("ident", [P, P], BF16, kind="ExternalInput").ap()
        maskT = nc.dram_tensor("maskT", [S, S], BF16, kind="ExternalInput").ap()
    out = nc.dram_tensor("out", [S, HID], BF16, kind="ExternalOutput").ap()

    def dr3(ps, l_hi, l_lo, r_hi, r_lo, kt_pairs, start, stop):
        """Accumulate (l_hi+l_lo).T@(r_hi+r_lo) (minus lo*lo) into `ps` via
        fp8 DoubleRow over `kt_pairs` 2-k-tile slices.  l_*/r_* are
        callables kt_pair_idx -> AP [128, 2, *]."""
        n = len(kt_pairs)
        terms = ((l_hi, r_hi), (l_hi, r_lo), (l_lo, r_hi))
        for j, t in enumerate(kt_pairs):
            for ti, (lsel, rsel) in enumerate(terms):
                nc.tensor.matmul(
                    ps,
                    lhsT=lsel(t),
                    rhs=rsel(t),
                    start=(start and j == 0 and ti == 0),
                    stop=(stop and j == n - 1 and ti == 2),
                    perf_mode=DR,
                )

    with tile.TileContext(nc) as tc:
        with (
            tc.tile_pool(name="resid", bufs=1) as resid,
            tc.tile_pool(name="hst", bufs=2) as hst_pool,
            tc.tile_pool(name="cs", bufs=2) as cs_pool,
            tc.tile_pool(name="rst", bufs=2) as rst_pool,
            tc.tile_pool(name="oT", bufs=2) as oT_pool,
            tc.tile_pool(name="probs", bufs=8) as probs_pool,
            tc.tile_pool(name="tmp", bufs=1) as tmp_pool,
            tc.tile_pool(name="rb", bufs=2) as rb_pool,
            tc.tile_pool(name="acc", bufs=2) as acc_pool,
            tc.tile_pool(name="mchunk", bufs=4) as mchunk_pool,
            tc.tile_pool(name="outsb", bufs=4) as outsb_pool,
            tc.tile_pool(name="wo", bufs=2) as wo_pool,
            tc.tile_pool(name="mm_ps", bufs=2, space="PSUM") as mm_ps,
            tc.tile_pool(name="sc_ps", bufs=3, space="PSUM") as sc_ps,
            tc.tile_pool(name="o_ps", bufs=3, space="PSUM") as o_ps,
        ):
            # ---- persistent tiles ----
            qT_hi = [
                resid.tile([P, 2, S], F8, tag=f"qTh{h}", name=f"qTh{h}")
                for h in range(HPC)
            ]
            qT_lo = [
                resid.tile([P, 2, S], F8, tag=f"qTl{h}", name=f"qTl{h}")
                for h in range(HPC)
            ]
            kT_hi = resid.tile([P, 2, S], F8, tag="kTh", name="kT_hi")
            kT_lo = resid.tile([P, 2, S], F8, tag="kTl", name="kT_lo")
            vt = [resid.tile([P, HEAD_DIM], FP16, tag=f"v{i}", name=f"v{i}") for i in range(NKS)]
            wkh_sl = resid.tile([P, NHT, HEAD_DIM], F8, tag="wkh", name="wkh_sl")
            wkl_sl = resid.tile([P, NHT, HEAD_DIM], F8, tag="wkl", name="wkl_sl")
            wvh_sl = resid.tile([P, NHT, HEAD_DIM], F8, tag="wvh", name="wvh_sl")
            wvl_sl = resid.tile([P, NHT, HEAD_DIM], F8, tag="wvl", name="wvl_sl")
            wqh_sl = resid.tile([P, NHT, FQ], F8, tag="wqh", name="wqh_sl")
            wql_sl = resid.tile([P, NHT, FQ], F8, tag="wql", name="wql_sl")
            if mask_mode in ("causal", "full"):
                id_t = resid.tile([P, P], BF16, tag="ident", name="id_t")
                nc.sync.dma_start(out=id_t, in_=ident)
            if mask_mode == "causal":
                mt_t = resid.tile([P, P], BF16, tag="mtri", name="mt_t")
                nc.sync.dma_start(out=mt_t, in_=mtri)

            def rope_quant_pair(ps0, ps1, hi_t, lo_t, sb, cs, sn):
                """rope the pair of d-half PSUMs (cos/sin tables carry the
                1/WS descale), then hi/lo-quantize into hi_t/lo_t
                [P, 2, S] fp8 at column range sb*QB:+QB.  hi on ACT, lo
                residual on DVE."""
                t0 = tmp_pool.tile([P, QB], F32, tag="t0", name="t0")
                t1 = tmp_pool.tile([P, QB], F32, tag="t1", name="t1")
                t2 = tmp_pool.tile([P, QB], F32, tag="t2", name="t2")
                t3 = tmp_pool.tile([P, QB], F32, tag="t3", name="t3")
                nc.vector.tensor_mul(t0, ps0, cs)
                nc.vector.tensor_mul(t3, ps0, sn)
                nc.vector.tensor_mul(t1, ps1, sn)
                nc.vector.tensor_mul(t2, ps1, cs)
                sl = slice(sb * QB, (sb + 1) * QB)
                r0 = rst_pool.tile([P, QB], BF16, tag="r0", name="r0")
                r1 = rst_pool.tile([P, QB], BF16, tag="r1", name="r1")
                nc.vector.tensor_sub(r0, t0, t1)
                nc.vector.tensor_add(r1, t2, t3)
                for fd, r in ((0, r0), (1, r1)):
                    nc.scalar.activation(hi_t[:, fd, sl], r, ACT_COPY)
                    nc.vector.tensor_sub(lo_t[:, fd, sl], r, hi_t[:, fd, sl])

            # phase 1 rotates matmul groups over ALL 8 PSUM banks (the
            # sc/o pools are idle until phase 2), so the PE can run several
            # projection groups ahead of the rope/quantize drain.
            _ps_pools = [(mm_ps, "mm"), (sc_ps, "sc"), (o_ps, "o")]
            _ps_seq = [_ps_pools[i % 3] for i in range(24)]
            _ps_i = [0]

            def next_ps():
                pool, tag = _ps_pools[_ps_i[0] % 3]
                _ps_i[0] += 1
                return pool.tile([P, QB], F32, tag=tag, name=f"ps_{tag}")

            # ================= phase 1: projections + rope =================
            for sb in range(NSB):
                ssl = slice(sb * QB, (sb + 1) * QB)
                hsh_sl = hst_pool.tile([P, NHT, QB], F8, tag="hsh", name="hsh_sl")
                hsl_sl = hst_pool.tile([P, NHT, QB], F8, tag="hsl", name="hsl_sl")
                for hq in range(4):
                    hsl4 = slice(4 * hq, 4 * hq + 4)
                    if sb == 0:
                        for w_sl, w_dram in ((wkh_sl, wk_hi), (wkl_sl, wk_lo)):
                            nc.sync.dma_start(
                                out=w_sl[:, hsl4, :],
                                in_=w_dram.rearrange("(t p) f -> p t f", p=P)[:, hsl4, :],
                            )
                    for h_sl, h_dram in ((hsh_sl, hsT_hi), (hsl_sl, hsT_lo)):
                        nc.sync.dma_start(
                            out=h_sl[:, hsl4, :],
                            in_=h_dram.rearrange("(t p) s -> p t s", p=P)[:, hsl4, ssl],
                        )
                # rope tables arrive per-sb (rotating), keeping startup DMA low
                cos_sl = cs_pool.tile([P, QB], F32, tag="cos", name="cos_sl")
                sin_sl = cs_pool.tile([P, QB], F32, tag="sin", name="sin_sl")
                nc.sync.dma_start(out=cos_sl, in_=cosT[:, ssl])
                nc.sync.dma_start(out=sin_sl, in_=sinT[:, ssl])
                if sb == 0:
                    for hq in range(4):
                        hsl4 = slice(4 * hq, 4 * hq + 4)
                        for w_sl, w_dram in ((wvh_sl, wv_hi), (wvl_sl, wv_lo)):
                            nc.sync.dma_start(
                                out=w_sl[:, hsl4, :],
                                in_=w_dram.rearrange("(t p) f -> p t f", p=P)[:, hsl4, :],
                            )
                    # deferred bulk load: behind the sb0 essentials
                    for hq in range(4):
                        hsl4 = slice(4 * hq, 4 * hq + 4)
                        for w_sl, w_dram in ((wqh_sl, wq_hi), (wql_sl, wq_lo)):
                            nc.sync.dma_start(
                                out=w_sl[:, hsl4, :],
                                in_=w_dram.rearrange("(t p) f -> p t f", p=P)[:, hsl4, :],
                            )
                kt_pairs = list(range(0, NHT, 2))

                def hs_hi_sel(t, _h=hsh_sl):
                    return _h[:, t : t + 2, :]

                def hs_lo_sel(t, _h=hsl_sl):
                    return _h[:, t : t + 2, :]

                # k^T (one kv head: 2 d-halves), with rope
                ps_k = []
                for fd in range(2):
                    ps = next_ps()
                    fsl = slice(fd * P, (fd + 1) * P)
                    dr3(
                        ps,
                        lambda t, _f=fsl: wkh_sl[:, t : t + 2, _f],
                        lambda t, _f=fsl: wkl_sl[:, t : t + 2, _f],
                        hs_hi_sel,
                        hs_lo_sel,
                        kt_pairs,
                        start=True,
                        stop=True,
                    )
                    ps_k.append(ps)
                rope_quant_pair(ps_k[0], ps_k[1], kT_hi, kT_lo, sb, cos_sl, sin_sl)
                # v (natural layout [s, d]); keeps the x64 from wv's host
                # pre-scale (cancels in softmax normalization)
                for s_sub in range(4):
                    ps = next_ps()
                    psl = slice(s_sub * P, (s_sub + 1) * P)
                    dr3(
                        ps[:, :HEAD_DIM],
                        lambda t, _p=psl: hsh_sl[:, t : t + 2, _p],
                        lambda t, _p=psl: hsl_sl[:, t : t + 2, _p],
                        lambda t: wvh_sl[:, t : t + 2, :],
                        lambda t: wvl_sl[:, t : t + 2, :],
                        kt_pairs,
                        start=True,
                        stop=True,
                    )
                    nc.any.tensor_copy(out=vt[sb * 4 + s_sub], in_=ps[:, :HEAD_DIM])
                # q^T (4 heads x 2 d-halves), with rope
                for h in range(HPC):
                    ps_q = []
                    for fd in range(2):
                        ft = 2 * h + fd
                        fsl = slice(ft * P, (ft + 1) * P)
                        ps = next_ps()
                        dr3(
                            ps,
                            lambda t, _f=fsl: wqh_sl[:, t : t + 2, _f],
                            lambda t, _f=fsl: wql_sl[:, t : t + 2, _f],
                            hs_hi_sel,
                            hs_lo_sel,
                            kt_pairs,
                            start=True,
                            stop=True,
                        )
                        ps_q.append(ps)
                    rope_quant_pair(ps_q[0], ps_q[1], qT_hi[h], qT_lo[h], sb, cos_sl, sin_sl)

            # ============ phase 2+3: attention + o_proj, per qs-block ============
            for qb in range(NSB):
                qsl = slice(qb * QB, (qb + 1) * QB)
                nks = 4 * qb + 4 if mask_mode == "causal" else NKS
                oTh_qb = oT_pool.tile([P, NFQ, QB], F8, tag="oTh", name="oTh")
                oTl_qb = oT_pool.tile([P, NFQ, QB], F8, tag="oTl", name="oTl")
                for h in range(HPC):
                    ps_o0 = o_ps.tile([P, QB], F32, tag="o", name="ps_o")
                    ps_o1 = o_ps.tile([P, QB], F32, tag="o", name="ps_o")
                    acc = acc_pool.tile([P, QB], FP16, tag="acc", name="acc")
                    for ks in range(nks):
                        ksl = slice(ks * P, (ks + 1) * P)
                        m = ks - 4 * qb if mask_mode == "causal" else -1
                        # columns [0, 128m) of a diagonal tile are fully
                        # masked -> compute only the live range [c0, QB)
                        c0 = 128 * m if m > 0 else 0
                        w = QB - c0
                        qslw = slice(qb * QB + c0, (qb + 1) * QB)
                        ps_s = sc_ps.tile([P, QB], F32, tag="sc", name="ps_s")
                        if m >= 0:
                            # triangle mask opens the accumulation group
                            # (start clears the bank; scores then overwrite
                            # the never-written columns, accumulate on the
                            # triangle ones)
                            nc.tensor.matmul(
                                ps_s[:, c0 : c0 + P],
                                lhsT=id_t,
                                rhs=mt_t,
                                start=True,
                                stop=False,
                            )
                        for pi, (kt, qt) in enumerate(
                            (
                                (kT_hi, qT_hi[h]),
                                (kT_hi, qT_lo[h]),
                                (kT_lo, qT_hi[h]),
                            )
                        ):
                            nc.tensor.matmul(
                                ps_s[:, c0:],
                                lhsT=kt[:, :, ksl],
                                rhs=qt[:, :, qslw],
                                start=(m < 0 and pi == 0),
                                stop=(mask_mode != "full" and pi == 2),
                                perf_mode=DR,
                            )
                        if mask_mode == "full":
                            mc = mchunk_pool.tile([P, QB], BF16, tag="mc", name="mc")
                            nc.sync.dma_start(out=mc, in_=maskT[ksl, qsl])
                            nc.tensor.matmul(
                                ps_s, lhsT=id_t, rhs=mc, start=False, stop=True
                            )
                        probs = probs_pool.tile([P, QB], FP16, tag="pr", name="probs")
                        nc.scalar.activation(
                            probs[:, :w], ps_s[:, c0:], ACT_EXP, scale=SCALE
                        )
                        nc.tensor.matmul(
                            ps_o0[:, c0:],
                            lhsT=vt[ks][:, :HD2],
                            rhs=probs[:, :w],
                            start=(ks == 0),
                            stop=(ks == nks - 1),
                        )
                        nc.tensor.matmul(
                            ps_o1[:, c0:],
                            lhsT=vt[ks][:, HD2:],
                            rhs=probs[:, :w],
                            start=(ks == 0),
                            stop=(ks == nks - 1),
                        )
                        # rowsum accumulation on DVE: all-fp16 operands hit
                        # the 2x_1p fast path (267ns vs 1.3us on GPSIMD)
                        if ks == 0:
                            nc.vector.tensor_copy(out=acc, in_=probs)
                        else:
                            nc.vector.tensor_add(
                                acc[:, c0:], acc[:, c0:], probs[:, :w]
                            )
                    # 1/colsum: partition all-reduce (broadcasts too), recip
                    zb = rb_pool.tile([P, QB], F32, tag="zb", name="zb")
                    nc.gpsimd.partition_all_reduce(
                        zb, acc, channels=P, reduce_op=bass_isa.ReduceOp.add
                    )
                    rb = rb_pool.tile([P, QB], F32, tag="rb", name="rb")
                    nc.vector.reciprocal_approx_fast(rb, zb)
                    # normalize + hi/lo quantize oT (x64 from v's pre-scale
                    # keeps the fp8 encoding clear of denormals)
                    for fd, ps_o in ((0, ps_o0), (1, ps_o1)):
                        f = 2 * h + fd
                        nt = tmp_pool.tile([P, QB], F32, tag=f"n{fd}", name="nt")
                        nc.vector.tensor_mul(nt, ps_o, rb)
                        nc.scalar.activation(oTh_qb[:, f, :], nt, ACT_COPY)
                        nc.vector.tensor_sub(oTl_qb[:, f, :], nt, oTh_qb[:, f, :])
                # ---- o_proj for this qs-block ----
                f_pairs = list(range(0, NFQ, 2))
                for hc in range(NSB):
                    hsl = slice(hc * QB, (hc + 1) * QB)
                    woh_sl = wo_pool.tile([P, NFQ, QB], F8, tag="woh", name="woh_sl")
                    wol_sl = wo_pool.tile([P, NFQ, QB], F8, tag="wol", name="wol_sl")
                    for w_sl, w_dram in ((woh_sl, wo_hi), (wol_sl, wo_lo)):
                        nc.sync.dma_start(
                            out=w_sl,
                            in_=w_dram.rearrange("(t p) h -> p t h", p=P)[:, :, hsl],
                        )
                    for s_sub in range(4):
                        psl = slice(s_sub * P, (s_sub + 1) * P)
                        ps = mm_ps.tile([P, QB], F32, tag="mm", name="ps_mm")
                        dr3(
                            ps,
                            lambda t, _p=psl: oTh_qb[:, t : t + 2, _p],
                            lambda t, _p=psl: oTl_qb[:, t : t + 2, _p],
                            lambda t: woh_sl[:, t : t + 2, :],
                            lambda t: wol_sl[:, t : t + 2, :],
                            f_pairs,
                            start=True,
                            stop=True,
                        )
                        ot = outsb_pool.tile([P, QB], BF16, tag="ot", name="ot")
                        # descale: wv's x32 (carried through oT) and wo's x64
                        nc.scalar.activation(
                            ot, ps, ACT_COPY, scale=1.0 / (WS * WSV)
                        )
                        nc.sync.dma_start(
                            out=out[qb * QB + s_sub * P : qb * QB + (s_sub + 1) * P, hsl],
                            in_=ot,
                        )
    nc.compile()
    return nc


_BUILD_CACHE: dict = {}


def _get_kernel(mask_mode: str):
    if mask_mode not in _BUILD_CACHE:
        _BUILD_CACHE[mask_mode] = _build(mask_mode)
    return _BUILD_CACHE[mask_mode]


def _rope_tables(position_ids_b: np.ndarray):
    """cos/sin half-tables, transposed [HD2, S] f32, carrying 1/WS."""
    inv_freq = (
        1.0 / (ROPE_BASE ** (np.arange(0, HEAD_DIM, 2, dtype=np.float32) / HEAD_DIM))
    ).astype(np.float32)
    freqs = position_ids_b.astype(np.float32)[:, None] * inv_freq[None, :]  # [S, HD2]
    return (
        np.ascontiguousarray((np.cos(freqs) / WS).astype(np.float32).T),
        np.ascontiguousarray((np.sin(freqs) / WS).astype(np.float32).T),
    )


def _split8(x: np.ndarray):
    hi = x.astype(NP_F8)
    lo = (x - hi.astype(np.float32)).astype(NP_F8)
    return hi, lo


def kernel(hidden_states, attention_mask, position_ids, Wq, Wk, Wv, Wo):
    hidden_states = np.asarray(hidden_states, dtype=np.float32)
    attention_mask = np.asarray(attention_mask, dtype=np.float32)
    position_ids = np.asarray(position_ids)
    Wq = np.asarray(Wq, dtype=np.float32)
    Wk = np.asarray(Wk, dtype=np.float32)
    Wv = np.asarray(Wv, dtype=np.float32)
    Wo = np.asarray(Wo, dtype=np.float32)

    # mask classification
    tri = np.tril(np.ones((S, S), dtype=bool))
    canonical = np.where(tri, np.float32(0.0), np.float32(-1e9))
    is_causal = all(
        np.array_equal(attention_mask[b, 0], canonical) for b in range(B)
    )
    if is_causal:
        mask_mode = "causal"
    elif not attention_mask.any():
        mask_mode = "none"
    else:
        mask_mode = "full"

    nc = _get_kernel(mask_mode)

    ident = np.eye(P, dtype=np.float32).astype(NP_BF16)
    if mask_mode == "causal":
        ii = np.arange(P)[:, None]
        jj = np.arange(P)[None, :]
        mtri = np.where(jj >= ii, np.float32(0.0), np.float32(-16e9)).astype(NP_BF16)

    in_maps = []
    for core in range(8):
        b, g = core // 4, core % 4
        hsT = np.ascontiguousarray(hidden_states[b].T)
        hsT_hi, hsT_lo = _split8(hsT)
        wq_hi, wq_lo = _split8(
            np.ascontiguousarray(Wq[:, g * FQ : (g + 1) * FQ]) * WS
        )
        wk_hi, wk_lo = _split8(
            np.ascontiguousarray(Wk[:, g * HEAD_DIM : (g + 1) * HEAD_DIM]) * WS
        )
        wv_hi, wv_lo = _split8(
            np.ascontiguousarray(Wv[:, g * HEAD_DIM : (g + 1) * HEAD_DIM]) * WSV
        )
        wo_hi, wo_lo = _split8(np.ascontiguousarray(Wo[g * FQ : (g + 1) * FQ, :]) * WS)
        m = {
            "hsT_hi": hsT_hi, "hsT_lo": hsT_lo,
            "wq_hi": wq_hi, "wq_lo": wq_lo,
            "wk_hi": wk_hi, "wk_lo": wk_lo,
            "wv_hi": wv_hi, "wv_lo": wv_lo,
            "wo_hi": wo_hi, "wo_lo": wo_lo,
        }
        cosT, sinT = _rope_tables(position_ids[b])
        m["cosT"], m["sinT"] = cosT, sinT
        if mask_mode == "causal":
            m["ident"] = ident
            m["mtri"] = mtri
        elif mask_mode == "full":
            m["ident"] = ident
            m["maskT"] = np.ascontiguousarray(
                (attention_mask[b, 0].T * np.float32(16.0))
            ).astype(NP_BF16)
        in_maps.append(m)

    global _LAST_IN_MAPS
    _LAST_IN_MAPS = in_maps
    res = run_bass_kernel_spmd(nc, in_maps, list(range(8)))
    outs = [res.results[c]["out"].astype(np.float32) for c in range(8)]
    full = np.empty((B, S, HID), dtype=np.float32)
    for b in range(B):
        full[b] = outs[4 * b] + outs[4 * b + 1] + outs[4 * b + 2] + outs[4 * b + 3]
    return full
